# revision 47
# baseline (speedup 1.0000x reference)
"""TRN2 Bass kernel for nn_BinarySCLoss_67207648248312.

Math (reference reformulation, validated to ~7e-7 rel err):
  The two unfold scales (3x3 and 5x5) share identical per-offset losses, so
  the 34 offset-terms collapse to 25 weighted offsets:
      c(o) = 1/9 + 0.5/25  for o in the inner 3x3,   0.5/25  for the outer ring.
  Pair symmetry: the denominator d(n, n+o) is symmetric in the pair, so
  offsets o and -o share one denominator:
      sum_n L_o[n] + sum_n L_{-o}[n] = sum_n (bce[n] + bce[n+o]) / d_o[n]
  leaving 12 pair-units + the center offset. Out-of-bounds (zero-padded)
  offsets contribute exactly 1.0 each; that is a pure geometry constant
  added on the host.

  Per pair o=(dy,dx):  joint = p*p_o ; g = joint - 2*(m*m_o), m = p*t
                       d = softplus(g) + exp(-q*q_o), q = sigmoid(p)
                       contrib = c(o) * (bce + bce_o) / d,  bce = softplus(p) - m
  (the +eps in the reference denominator is dropped: denominator >= exp(-1),
   so the relative effect is < 3e-6).

Layout: one 512x512 image per core (B=8 across 8 cores). SBUF maps are
[128, 2048]: partition p, column k*512+x holds pixel (y=128k+p, x).  dy-shifts
are SBUF->SBUF DMA copies (bulk partition shift + block-wrap columns); dx
shifts are free-dim offsets.

Default path (USE_BF16): per-pair products/subs/final-mul run on DVE in the
2-elem/cycle bf16 mode, softplus/sigmoid/exp chains on ACT from one activation
table (exp+ln, zero table reloads), the d/bsum adds on POOL, and the free-dim
reduction rides the ACT accumulator (activation Identity + accum_out) into
per-chunk acc columns; weights and the final sum are applied host-side
(slot_weights()).  Measured vs the f32 jax reference on trn2: rel err ~1.4e-4
(bf16 rounding, unbiased to ~1e-4 after the 2M-element mean).  The f32 body
(bf16=False, rel err ~3e-7, ~10% slower) is kept as a fallback.
"""

import numpy as np

H = 512
W = 512
NB = 4                 # row blocks of 128
COLS = NB * W          # 2048
P128 = 128
ACC_COLS = 64
W_INNER = 1.0 / 9.0 + 0.5 / 25.0
W_OUTER = 0.5 / 25.0
# Representatives of the 12 symmetric offset pairs (dy > 0, or dy==0 and dx > 0).
PAIRS = [
    (0, 1), (0, 2),
    (1, -2), (1, -1), (1, 0), (1, 1), (1, 2),
    (2, -2), (2, -1), (2, 0), (2, 1), (2, 2),
]

_NC_CACHE = None


def _weight(dy, dx):
    return W_INNER if (abs(dy) <= 1 and abs(dx) <= 1) else W_OUTER


def _oob_const(n_images):
    # Padding neighbors contribute loss == 1.0; pure geometry.
    tot = 0.0
    for dy in range(-2, 3):
        for dx in range(-2, 3):
            n_oob = H * W - (H - abs(dy)) * (W - abs(dx))
            tot += _weight(dy, dx) * n_oob
    return tot * n_images


def _patch_single_act_table():
    """All our ACT funcs (Exp, Ln) coexist in natural_log_exp_and_others,
    but the greedy table chooser maps Exp to exp_and_others and Ln to
    natural_log, reloading the ~1.3us table between almost every pair of
    activations.  Blank every other set (indices preserved -> correct
    act_func_set_id) so one load serves the whole kernel."""
    from concourse import bacc
    from concourse.hw_specs import get_activation_tables as real_gat

    if getattr(bacc.get_activation_tables, "_single_table", False):
        return

    def single(arch):
        tabs = real_gat(arch)
        return {
            k: (v if k == "natural_log_exp_and_others" else set())
            for k, v in tabs.items()
        }

    single._single_table = True
    bacc.get_activation_tables = single


def build_nc(stage=99, repeat=1, bf16=False):
    from concourse import bacc, tile
    import concourse.mybir as mybir

    _patch_single_act_table()

    F32 = mybir.dt.float32
    AF = mybir.ActivationFunctionType
    ALU = mybir.AluOpType

    nc = bacc.Bacc("TRN2", target_bir_lowering=False, debug=False)
    pred_d = nc.declare_dram_parameter("pred", [H, W], F32, isOutput=False)
    targ_d = nc.declare_dram_parameter("target", [H, W], F32, isOutput=False)
    out_d = nc.declare_dram_parameter("out", [P128, ACC_COLS], F32, isOutput=True)

    def blocked(ap):
        # [128, COLS] (or sliced partitions) -> [P, NB, W]
        return ap.rearrange("p (k x) -> p k x", x=W)

    def body(tc, maps_pool, tmp_pool, misc_pool):
            p_m = maps_pool.tile([P128, COLS], F32, tag="p")
            m_m = maps_pool.tile([P128, COLS], F32, tag="m")
            q_m = maps_pool.tile([P128, COLS], F32, tag="q")
            b_m = maps_pool.tile([P128, COLS], F32, tag="bce")
            m2_0 = maps_pool.tile([P128, COLS], F32, tag="m2_0")
            shifted = {}
            for nm in ("p", "m", "q", "bce"):
                for dy in (1, 2):
                    shifted[(nm, dy)] = maps_pool.tile(
                        [P128, COLS], F32, tag=f"{nm}_dy{dy}",
                        name=f"{nm}_dy{dy}")

            acc = misc_pool.tile([P128, ACC_COLS], F32, tag="acc")
            trash = misc_pool.tile([P128, COLS], F32, tag="trash")

            nc.vector.memset(acc[:, :], 0.0)

            # ---- load inputs ----
            nc.sync.dma_start(
                out=blocked(p_m[:, :]),
                in_=pred_d.ap().rearrange("(k p) x -> p k x", p=P128))
            # target parks in `trash` until folded into m = p*t
            nc.sync.dma_start(
                out=blocked(trash[:, :]),
                in_=targ_d.ap().rearrange("(k p) x -> p k x", p=P128))

            def anchor(tl, slot, pc=P128):
                # keep a tile live through DCE: reduce it into an acc column
                nc.vector.reduce_sum(
                    out=acc[0:pc, slot:slot + 1],
                    in_=tl[0:pc, :], axis=mybir.AxisListType.X)

            if stage <= 1:
                anchor(p_m, 60)
                anchor(trash, 61)
                nc.sync.dma_start(out=out_d.ap(), in_=acc[:, :])
                return

            # ---- per-pixel precompute ----
            # Only exp/ln are used anywhere (single ACT table, no reloads):
            #   spn = softplus(-p) = ln(exp(-p) + 1)
            #   q   = sigmoid(p)   = exp(-spn)
            #   bce = softplus(p) - p*t = spn + p*(1 - t) = spn + (p - m)
            nc.vector.tensor_tensor(
                out=m_m[:, :], in0=p_m[:, :], in1=trash[:, :], op=ALU.mult)
            spn = tmp_pool.tile([P128, COLS], F32, tag="sp")
            nc.scalar.activation(
                out=spn[:, :], in_=p_m[:, :], func=AF.Exp, scale=-1.0)
            nc.scalar.activation(
                out=spn[:, :], in_=spn[:, :], func=AF.Ln, bias=1.0)
            nc.scalar.activation(
                out=q_m[:, :], in_=spn[:, :], func=AF.Exp, scale=-1.0)
            u0 = tmp_pool.tile([P128, COLS], F32, tag="prod")
            nc.vector.tensor_tensor(
                out=u0[:, :], in0=p_m[:, :], in1=m_m[:, :], op=ALU.subtract)
            nc.vector.tensor_tensor(
                out=b_m[:, :], in0=spn[:, :], in1=u0[:, :], op=ALU.add)

            if stage <= 2:
                anchor(m_m, 56)
                anchor(q_m, 57)
                anchor(b_m, 58)
                nc.sync.dma_start(out=out_d.ap(), in_=acc[:, :])
                return

            # The "m2" operand of each pair is 2*m shifted by dy; double once,
            # then shift the doubled map (g = joint - m*m2_shift needs no
            # scaled op, and plain tensor_tensor is the only fused-free ISA
            # form that walrus accepts here).
            nc.vector.tensor_tensor(
                out=m2_0[:, :], in0=m_m[:, :], in1=m_m[:, :], op=ALU.add)

            # ---- dy-shifted copies (SBUF->SBUF DMA) ----
            base = {"p": p_m, "m": m2_0, "q": q_m, "bce": b_m}
            for (nm, dy), dst in shifted.items():
                src = base[nm]
                # rows y+dy for y in-block: partition shift
                nc.sync.dma_start(
                    out=dst[0:P128 - dy, :], in_=src[dy:P128, :])
                # block-wrap rows: partition 128-dy+j of block k = row j of block k+1
                nc.sync.dma_start(
                    out=dst[P128 - dy:P128, 0:(NB - 1) * W],
                    in_=src[0:dy, W:COLS])

            # ---- pair units ----
            def pair_unit(dy, dx, slot):
                wgt = _weight(dy, dx)
                Wp = W - abs(dx)
                xc = max(0, -dx)       # center window column offset
                xs = max(0, dx)        # shifted-operand window column offset
                center = dy == 0 and dx == 0
                if dy == 0:
                    chunks = [(P128, 0, NB)]
                else:
                    chunks = [(P128, 0, NB - 1), (P128 - dy, NB - 1, 1)]

                if dy == 0:
                    p_s, m_s, q_s, b_s = p_m, m2_0, q_m, b_m
                else:
                    p_s = shifted[("p", dy)]
                    m_s = shifted[("m", dy)]   # holds 2*m(y+dy)
                    q_s = shifted[("q", dy)]
                    b_s = shifted[("bce", dy)]

                joint = tmp_pool.tile([P128, COLS], F32, tag="joint")
                mm = tmp_pool.tile([P128, COLS], F32, tag="mm")
                sp = tmp_pool.tile([P128, COLS], F32, tag="sp")
                prod = tmp_pool.tile([P128, COLS], F32, tag="prod")
                bs = None if center else tmp_pool.tile(
                    [P128, COLS], F32, tag="bs")

                for pc, k0, kn in chunks:
                    def C(tl, xo=xc):
                        return blocked(tl[0:pc, :])[:, k0:k0 + kn, xo:xo + Wp]

                    # joint = p * p_shift      (POOL)
                    nc.gpsimd.tensor_tensor(
                        out=C(joint), in0=C(p_m), in1=C(p_s, xs), op=ALU.mult)
                    # mm = m * (2*m)_shift     (POOL)
                    nc.gpsimd.tensor_tensor(
                        out=C(mm), in0=C(m_m), in1=C(m_s, xs), op=ALU.mult)
                    if stage == 41:
                        anchor(joint, 40, pc=pc)
                        anchor(mm, 41, pc=pc)
                        slot += 1
                        continue
                    # g = joint - mm     (in place over joint)
                    nc.vector.tensor_tensor(
                        out=C(joint), in0=C(joint), in1=C(mm), op=ALU.subtract)
                    if stage == 42:
                        anchor(joint, 40, pc=pc)
                        slot += 1
                        continue
                    # softplus(g) = ln(exp(g) + 1); our g is bounded (~|27|)
                    nc.scalar.activation(
                        out=C(sp), in_=C(joint), func=AF.Exp)
                    nc.scalar.activation(
                        out=C(sp), in_=C(sp), func=AF.Ln, bias=1.0)
                    if stage == 43:
                        anchor(sp, 40, pc=pc)
                        slot += 1
                        continue
                    # prod = q * q_shift
                    nc.vector.tensor_tensor(
                        out=C(prod), in0=C(q_m), in1=C(q_s, xs), op=ALU.mult)
                    # pw = exp(-prod)   (in place)
                    nc.scalar.activation(
                        out=C(prod), in_=C(prod), func=AF.Exp, scale=-1.0)
                    # d = sp + pw       (in place over sp)
                    nc.vector.tensor_tensor(
                        out=C(sp), in0=C(sp), in1=C(prod), op=ALU.add)
                    if stage == 44:
                        anchor(sp, 40, pc=pc)
                        slot += 1
                        continue
                    # r = 1/d           (in place; DVE divide doesn't exist)
                    nc.vector.reciprocal(out=C(sp), in_=C(sp))
                    if stage == 45:
                        anchor(sp, 40, pc=pc)
                        slot += 1
                        continue
                    if center:
                        num = C(b_m)
                    else:
                        # bsum on POOL for dy>0 pairs to balance engines
                        eng = nc.gpsimd if dy > 0 else nc.vector
                        eng.tensor_tensor(
                            out=C(bs), in0=C(b_m), in1=C(b_s, xs), op=ALU.add)
                        num = C(bs)
                    # acc[slot] = sum( wgt * num * r )  (custom DVE op --
                    # InstTensorTensorReduce crashes the exec unit at runtime)
                    nc.vector.affine_mul_reduce(
                        out=C(trash), accum_out=acc[0:pc, slot:slot + 1],
                        in0=num, in1=C(sp), scale=wgt, bias=0.0)
                    slot += 1
                return slot

            if stage <= 3:
                for i, tl in enumerate(shifted.values()):
                    anchor(tl, 48 + i, pc=96)
                nc.sync.dma_start(out=out_d.ap(), in_=acc[:, :])
                return


            if stage == 36:
                jt = tmp_pool.tile([P128, COLS], F32, tag="joint")
                nc.gpsimd.tensor_tensor(
                    out=jt[0:126, :], in0=p_m[0:126, :], in1=q_m[0:126, :],
                    op=ALU.mult)
                anchor(jt, 40, pc=126)
                nc.sync.dma_start(out=out_d.ap(), in_=acc[:, :])
                return
            if stage == 37:
                jt = tmp_pool.tile([P128, COLS], F32, tag="joint")
                nc.gpsimd.tensor_tensor(
                    out=jt[:, :], in0=p_m[:, :], in1=p_m[:, :], op=ALU.mult)
                anchor(jt, 40)
                nc.sync.dma_start(out=out_d.ap(), in_=acc[:, :])
                return
            if stage == 38:
                jt = tmp_pool.tile([P128, COLS], F32, tag="joint")
                nc.gpsimd.tensor_tensor(
                    out=blocked(jt[:, :])[:, 0:3, 0:500],
                    in0=blocked(p_m[:, :])[:, 0:3, 0:500],
                    in1=blocked(q_m[:, :])[:, 0:3, 12:512], op=ALU.mult)
                anchor(acc, 40, pc=1)
                nc.sync.dma_start(out=out_d.ap(), in_=acc[:, :])
                return
            if stage == 39:
                jt = tmp_pool.tile([P128, COLS], F32, tag="joint")
                nc.gpsimd.tensor_tensor(
                    out=jt[:, :], in0=p_m[:, :], in1=q_m[:, :], op=ALU.mult)
                anchor(jt, 40)
                nc.sync.dma_start(out=out_d.ap(), in_=acc[:, :])
                return
            if stage == 40:
                jt = tmp_pool.tile([P128, COLS], F32, tag="joint")
                nc.vector.tensor_tensor(
                    out=blocked(jt[:, :])[:, 0:4, 0:512],
                    in0=blocked(p_m[:, :])[:, 0:4, 0:512],
                    in1=blocked(q_m[:, :])[:, 0:4, 0:512], op=ALU.mult)
                anchor(jt, 40)
                nc.sync.dma_start(out=out_d.ap(), in_=acc[:, :])
                return

            eff = 4 if 36 <= stage <= 46 else stage
            slot = 0
            slot = pair_unit(0, 0, slot)
            for dy, dx in PAIRS:
                if (dy == 0 and eff >= 5) or (dy == 1 and eff >= 6) \
                        or (dy == 2 and eff >= 7):
                    slot = pair_unit(dy, dx, slot)
            assert slot <= ACC_COLS

            nc.sync.dma_start(out=out_d.ap(), in_=acc[:, :])

    BF16 = mybir.dt.bfloat16

    def body16(tc, maps_pool, tmp_pool, misc_pool):
        # bf16 variant (V2): products/subs/final mul run on DVE in the
        # 2-elem/cycle bf16 mode; the d/bsum adds alternate DVE/POOL; the
        # reduce is a stock bf16 tensor_reduce (weights applied host-side,
        # see slot_weights()).  Only bf16 maps persist (52KB/partition).
        p16 = maps_pool.tile([P128, COLS], BF16, tag="p16")
        m16 = maps_pool.tile([P128, COLS], BF16, tag="m16")
        m2_16 = maps_pool.tile([P128, COLS], BF16, tag="m2_16")
        q16 = maps_pool.tile([P128, COLS], BF16, tag="q16")
        b16 = maps_pool.tile([P128, COLS], BF16, tag="b16")
        shifted = {}
        for nm in ("p16", "m2_16", "q16", "b16"):
            for dy in (1, 2):
                shifted[(nm, dy)] = maps_pool.tile(
                    [P128, COLS], BF16, tag=f"{nm}_dy{dy}",
                    name=f"{nm}_dy{dy}")

        acc = misc_pool.tile([P128, ACC_COLS], F32, tag="acc")
        p_f = misc_pool.tile([P128, COLS], F32, tag="p_f")
        t_f = misc_pool.tile([P128, COLS], F32, tag="t_f")
        m_f = misc_pool.tile([P128, COLS], F32, tag="m_f")
        nc.vector.memset(acc[:, :], 0.0)

        nc.sync.dma_start(
            out=blocked(p_f[:, :]),
            in_=pred_d.ap().rearrange("(k p) x -> p k x", p=P128))
        nc.sync.dma_start(
            out=blocked(t_f[:, :]),
            in_=targ_d.ap().rearrange("(k p) x -> p k x", p=P128))

        # precompute (see f32 body for the exp/ln-only identities)
        nc.vector.tensor_tensor(
            out=m_f[:, :], in0=p_f[:, :], in1=t_f[:, :], op=ALU.mult)
        spn = tmp_pool.tile([P128, COLS], F32, tag="sp")
        nc.scalar.activation(
            out=spn[:, :], in_=p_f[:, :], func=AF.Exp, scale=-1.0)
        nc.scalar.activation(
            out=spn[:, :], in_=spn[:, :], func=AF.Ln, bias=1.0)
        nc.scalar.activation(
            out=q16[:, :], in_=spn[:, :], func=AF.Exp, scale=-1.0)
        # u0 = p - m reuses the target tile (dead after m)
        nc.vector.tensor_tensor(
            out=t_f[:, :], in0=p_f[:, :], in1=m_f[:, :], op=ALU.subtract)
        nc.vector.tensor_tensor(
            out=b16[:, :], in0=spn[:, :], in1=t_f[:, :], op=ALU.add)
        nc.vector.tensor_copy(p16[:, :], p_f[:, :])
        nc.vector.tensor_copy(m16[:, :], m_f[:, :])
        nc.vector.tensor_tensor(
            out=m2_16[:, :], in0=m16[:, :], in1=m16[:, :], op=ALU.add)

        base = {"p16": p16, "m2_16": m2_16, "q16": q16, "b16": b16}
        for (nm, dy), dst in shifted.items():
            src = base[nm]
            nc.sync.dma_start(out=dst[0:P128 - dy, :], in_=src[dy:P128, :])
            nc.sync.dma_start(
                out=dst[P128 - dy:P128, 0:(NB - 1) * W],
                in_=src[0:dy, W:COLS])

        def pair_unit(dy, dx, slot, d_on_pool):
            Wp = W - abs(dx)
            xc = max(0, -dx)
            xs = max(0, dx)
            center = dy == 0 and dx == 0
            if dy == 0:
                chunks = [(P128, 0, NB)]
            else:
                chunks = [(P128, 0, NB - 1), (P128 - dy, NB - 1, 1)]
            if dy == 0:
                p_s, m_s, q_s, b_s = p16, m2_16, q16, b16
            else:
                p_s = shifted[("p16", dy)]
                m_s = shifted[("m2_16", dy)]
                q_s = shifted[("q16", dy)]
                b_s = shifted[("b16", dy)]

            for pc, k0, kn in chunks:
                joint = tmp_pool.tile([P128, COLS], BF16, tag="joint")
                mm = tmp_pool.tile([P128, COLS], BF16, tag="mm")
                prod = tmp_pool.tile([P128, COLS], BF16, tag="prod")
                sp = tmp_pool.tile([P128, COLS], BF16, tag="sp")
                r16 = tmp_pool.tile([P128, COLS], BF16, tag="r16")
                bs = None if center else tmp_pool.tile(
                    [P128, COLS], BF16, tag="bs")
                t16 = tmp_pool.tile([P128, COLS], BF16, tag="t16")
                def C(tl, xo=xc):
                    return blocked(tl[0:pc, :])[:, k0:k0 + kn, xo:xo + Wp]

                nc.vector.tensor_tensor(
                    out=C(joint), in0=C(p16), in1=C(p_s, xs), op=ALU.mult)
                nc.vector.tensor_tensor(
                    out=C(mm), in0=C(m16), in1=C(m_s, xs), op=ALU.mult)
                nc.vector.tensor_tensor(
                    out=C(joint), in0=C(joint), in1=C(mm), op=ALU.subtract)
                nc.scalar.activation(
                    out=C(sp), in_=C(joint), func=AF.Exp)
                nc.scalar.activation(
                    out=C(sp), in_=C(sp), func=AF.Ln, bias=1.0)
                nc.vector.tensor_tensor(
                    out=C(prod), in0=C(q16), in1=C(q_s, xs), op=ALU.mult)
                nc.scalar.activation(
                    out=C(prod), in_=C(prod), func=AF.Exp, scale=-1.0)
                # d = sp + pw
                nc.gpsimd.tensor_tensor(
                    out=C(sp), in0=C(sp), in1=C(prod), op=ALU.add)
                with nc.allow_low_precision("r rounds to bf16; final sum "
                                            "averages 2M terms"):
                    nc.vector.reciprocal(out=C(r16), in_=C(sp))
                if center:
                    num = C(b16)
                else:
                    nc.gpsimd.tensor_tensor(
                        out=C(bs), in0=C(b16), in1=C(b_s, xs), op=ALU.add)
                    num = C(bs)
                nc.vector.tensor_tensor(
                    out=C(t16), in0=num, in1=C(r16), op=ALU.mult)
                # free-dim reduction on the (underloaded) ACT engine
                nc.scalar.activation(
                    out=C(t16), in_=C(t16), func=AF.Identity,
                    accum_out=acc[0:pc, slot:slot + 1])
                slot += 1
            return slot

        slot = 0
        slot = pair_unit(0, 0, slot, d_on_pool=False)
        for i, (dy, dx) in enumerate(PAIRS):
            slot = pair_unit(dy, dx, slot, d_on_pool=(i % 2 == 0))
        assert slot <= ACC_COLS
        nc.sync.dma_start(out=out_d.ap(), in_=acc[:, :])

    with tile.TileContext(nc) as tc:
        with tc.tile_pool(name="maps", bufs=1) as maps_pool, \
             tc.tile_pool(name="tmp", bufs=(4 if bf16 else 2)) as tmp_pool, \
             tc.tile_pool(name="misc", bufs=1) as misc_pool:
            for _ in range(repeat):
                if bf16:
                    body16(tc, maps_pool, tmp_pool, misc_pool)
                else:
                    body(tc, maps_pool, tmp_pool, misc_pool)

    nc.compile()
    return nc


def _get_nc():
    global _NC_CACHE
    if _NC_CACHE is None:
        _NC_CACHE = build_nc(bf16=USE_BF16)
    return _NC_CACHE


USE_BF16 = True


def slot_weights():
    """acc-column weights, mirroring the build order of pair units.
    The f32 body applies weights on-chip (affine_mul_reduce scale); the
    bf16 body's stock reduce does not, so the host applies them here."""
    w = np.zeros(ACC_COLS)
    s = 0
    for dy, dx in [(0, 0)] + PAIRS:
        for _ in range(1 if dy == 0 else 2):
            w[s] = _weight(dy, dx) if USE_BF16 else 1.0
            s += 1
    if not USE_BF16:
        w[:s] = 1.0
    return w


def kernel(pred, target):
    from concourse.bass_utils import run_bass_kernel_spmd

    pred = np.asarray(pred)
    target = np.asarray(target)
    B = pred.shape[0]
    assert pred.shape == (B, 1, H, W)
    nc = _get_nc()
    in_maps = [
        {
            "pred": np.ascontiguousarray(pred[b, 0], dtype=np.float32),
            "target": np.ascontiguousarray(target[b, 0], dtype=np.float32),
        }
        for b in range(B)
    ]
    res = run_bass_kernel_spmd(nc, in_maps, list(range(B)))
    w = slot_weights()
    total = _oob_const(B)
    for r in res.results:
        cols = np.sum(np.asarray(r["out"], dtype=np.float64), axis=0)
        total += float(np.dot(cols, w))
    return np.float32(total / (B * H * W))
